# revision 7
# baseline (speedup 1.0000x reference)
"""Distributed multi-head attention kernel for one TRN2 chip (8 NeuronCores).

Problem: B=4, S=2048, D=1024, H=16, Dh=64 fp32 attention
    q,k,v = x@W* + b*  (per head)  ->  softmax(q k^T / sqrt(Dh)) v  -> @Wo + bo

Sharding (per the hint): data-parallel over B (4) x tensor-parallel over
head-halves (2) = 8 cores.  Core c = 2*b + hg handles batch b and heads
[8*hg, 8*hg+8) i.e. d_model slice [512*hg, 512*hg+512).  Each core produces
a partial output [2048, 1024] (its 8 heads' contribution through Wo); the
host sums the two partials per batch and adds bo (the unshard step).

Per-core layout trick: everything runs in "features-on-partitions" space.
The host supplies x[b] pre-transposed (xT [1024, 2048]) so Q^T and K^T come
out of the QKV matmuls directly as [d, s], which makes scores^T = K^T.T @ ...
land as [k_seq, q_seq] tiles with k on partitions.  Softmax over k (the
partition axis) is never done directly: we exp() unnormalized, and compute
both ctx^T and the row-sums r in ONE matmul by augmenting V with a block of
ones columns (out rows 0-63 = ctx^T, rows 64-127 = r replicated).  The
normalization (divide by r) happens on ctx^T (64 x 512 tiles), cheap.
Per-head normalization must precede the Wo projection (heads mix there).

Compute dtype: bf16 operands, fp32 PSUM accumulate (rel-err ~1e-3, well
under the 2e-2 gate).  All matmuls run at 1 cycle/row on the PE.
"""

import sys

sys.path.insert(0, "/opt/trn_rl_repo")

import numpy as np
import ml_dtypes

from contextlib import ExitStack

import concourse.bass as bass
import concourse.tile as tile
from concourse import bacc, mybir
from concourse.bass_utils import run_bass_kernel_spmd

BF16 = mybir.dt.bfloat16
F32 = mybir.dt.float32
AF = mybir.ActivationFunctionType


def _install_ntff_hook():
    """Provide antenv.axon_hooks (missing in this image) so that
    run_bass_kernel_spmd(trace=True) can capture NTFF profiles via the
    axon PJRT .so's C ABI."""
    import types, ctypes, contextlib

    if "antenv.axon_hooks" in sys.modules:
        return
    so_path = "/opt/axon/libaxon_pjrt.so"
    mod = types.ModuleType("antenv.axon_hooks")
    _state = {"hook": None}

    def set_axon_ntff_profile_hook(h):
        _state["hook"] = h

    def get_axon_ntff_profile_hook():
        return _state["hook"]

    mod.set_axon_ntff_profile_hook = set_axon_ntff_profile_hook
    mod.get_axon_ntff_profile_hook = get_axon_ntff_profile_hook
    sys.modules["antenv.axon_hooks"] = mod
    import antenv

    antenv.axon_hooks = mod

    try:
        lib = ctypes.CDLL(so_path)
    except OSError:
        return
    if not hasattr(lib, "axon_start_nrt_profile"):
        return
    lib.axon_start_nrt_profile.argtypes = [
        ctypes.POINTER(ctypes.c_int64),
        ctypes.c_size_t,
    ]
    lib.axon_start_nrt_profile.restype = ctypes.c_int64
    lib.axon_stop_nrt_profile.argtypes = [ctypes.c_char_p]
    lib.axon_stop_nrt_profile.restype = ctypes.c_int64

    @contextlib.contextmanager
    def _hook(output_dir, device_ids):
        import jax

        jax.devices()
        if device_ids:
            ids = (ctypes.c_int64 * len(device_ids))(*device_ids)
            rc = lib.axon_start_nrt_profile(ids, len(device_ids))
        else:
            rc = lib.axon_start_nrt_profile(None, 0)
        if rc != 0:
            raise RuntimeError(f"axon_start_nrt_profile rc={rc}")
        try:
            yield
        finally:
            n = lib.axon_stop_nrt_profile(str(output_dir).encode())
            print(f"profile: {n} file(s) written to {output_dir}",
                  file=sys.stderr)

    set_axon_ntff_profile_hook(_hook)


_install_ntff_hook()

D = 1024          # d_model
DC = 512          # per-core d slice (8 heads)
H_CORE = 8        # heads per core
DH = 64           # head dim
NPAIRS = 4        # head pairs per core


def build_graph(S=2048, trace_scopes=False):
    """Build the per-core Bass graph (same graph on all 8 cores)."""
    nc = bacc.Bacc(
        "TRN2",
        target_bir_lowering=False,
        debug=False,
        enable_asserts=False,
        num_devices=8,
    )

    ST = S // 128       # number of 128-seq tiles (16)
    QT_ = S // 512      # number of 512-seq tiles (4)

    xT = nc.dram_tensor("xT", [D, S], BF16, kind="ExternalInput").ap()
    wq = nc.dram_tensor("wq", [D, DC], BF16, kind="ExternalInput").ap()
    wk = nc.dram_tensor("wk", [D, DC], BF16, kind="ExternalInput").ap()
    wv = nc.dram_tensor("wv", [D, DC], BF16, kind="ExternalInput").ap()
    wo = nc.dram_tensor("wo", [DC, D], BF16, kind="ExternalInput").ap()
    bq = nc.dram_tensor("bq", [DC, 1], F32, kind="ExternalInput").ap()
    bk = nc.dram_tensor("bk", [DC, 1], F32, kind="ExternalInput").ap()
    bvr = nc.dram_tensor("bvr", [1, DC], BF16, kind="ExternalInput").ap()
    ones = nc.dram_tensor("ones", [1, 128], BF16, kind="ExternalInput").ap()
    out = nc.dram_tensor("out", [S, D], F32, kind="ExternalOutput").ap()

    with tile.TileContext(nc) as tc, ExitStack() as ctx:
        # ---- persistent pools --------------------------------------------
        qt_pool = ctx.enter_context(tc.tile_pool(name="qt", bufs=4))
        kt_pool = ctx.enter_context(tc.tile_pool(name="kt", bufs=4))
        vaug_pool = ctx.enter_context(tc.tile_pool(name="vaug", bufs=ST))
        ctx_pool = ctx.enter_context(tc.tile_pool(name="ctxT", bufs=4))
        const_pool = ctx.enter_context(tc.tile_pool(name="consts", bufs=1))
        wo_pool = ctx.enter_context(tc.tile_pool(name="wo", bufs=4))

        ones_sb = const_pool.tile([1, 128], BF16, tag="ones")
        nc.sync.dma_start(ones_sb[:], ones[:])
        bvr_sb = const_pool.tile([1, DC], BF16, tag="bvr")
        nc.sync.dma_start(bvr_sb[:], bvr[:])
        bq_sb = const_pool.tile([128, DC // 128], F32, tag="bq")
        nc.sync.dma_start(bq_sb[:], bq.rearrange("(m p) o -> p (m o)", p=128))
        bk_sb = const_pool.tile([128, DC // 128], F32, tag="bk")
        nc.sync.dma_start(bk_sb[:], bk.rearrange("(m p) o -> p (m o)", p=128))

        wo_tiles = []
        for dt_ in range(4):
            t = wo_pool.tile([128, D], BF16, tag="wo")
            nc.sync.dma_start(t[:], wo[dt_ * 128:(dt_ + 1) * 128, :])
            wo_tiles.append(t)

        qt_tiles = [qt_pool.tile([128, S], BF16, tag="qt", name=f"qt{i}") for i in range(4)]
        kt_tiles = [kt_pool.tile([128, S], BF16, tag="kt", name=f"ktt{i}") for i in range(4)]
        vaug_tiles = [vaug_pool.tile([128, H_CORE * 128], BF16, tag="vaug", name=f"vaug{i}")
                      for i in range(ST)]
        ctx_tiles = [ctx_pool.tile([128, S], BF16, tag="ctxT", name=f"ctxT{i}") for i in range(4)]

        # ---- phase B: projections ----------------------------------------
        with tc.tile_pool(name="xt", bufs=8) as xt_pool, \
             tc.tile_pool(name="wqkv", bufs=24) as w_pool, \
             tc.tile_pool(name="psB", bufs=4, space="PSUM") as psB:

            xt_tiles = []
            for kt_ in range(8):
                t = xt_pool.tile([128, S], BF16, tag="xt")
                nc.sync.dma_start(t[:], xT[kt_ * 128:(kt_ + 1) * 128, :])
                xt_tiles.append(t)

            wq_tiles, wk_tiles, wv_tiles = [], [], []
            for w_ap, lst, tag in ((wq, wq_tiles, "wq"), (wk, wk_tiles, "wk"),
                                   (wv, wv_tiles, "wv")):
                for kt_ in range(8):
                    t = w_pool.tile([128, DC], BF16, tag="wqkv")
                    nc.sync.dma_start(t[:], w_ap[kt_ * 128:(kt_ + 1) * 128, :])
                    lst.append(t)

            # V in natural [s, d] layout, into Vaug (64 V cols + 64 ones
            # cols per head block); bias via a K=1 ones-row matmul.
            for st in range(ST):
                pv = psB.tile([128, DC], F32, tag="psB")
                for kt_ in range(8):
                    nc.tensor.matmul(
                        pv[:],
                        xt_tiles[kt_][:, st * 128:(st + 1) * 128],
                        wv_tiles[kt_][:],
                        start=(kt_ == 0), stop=False,
                    )
                nc.tensor.matmul(pv[:], ones_sb[:], bvr_sb[:],
                                 start=False, stop=True)
                vt = vaug_tiles[st]
                nc.vector.memset(vt[:], 1.0)
                nc.vector.tensor_copy(
                    vt[:].rearrange("p (h w) -> p h w", h=H_CORE)[:, :, 0:64],
                    pv[:].rearrange("p (h w) -> p h w", h=H_CORE),
                )

            # Q^T and K^T in [d, s] layout; bias fused into the PSUM->SBUF
            # evacuation on DVE (per-partition scalar add).
            for (w_tiles, dst, b_sb) in ((wq_tiles, qt_tiles, bq_sb),
                                         (wk_tiles, kt_tiles, bk_sb)):
                for m in range(4):
                    for n in range(QT_):
                        p = psB.tile([128, 512], F32, tag="psB")
                        for kt_ in range(8):
                            nc.tensor.matmul(
                                p[:],
                                w_tiles[kt_][:, m * 128:(m + 1) * 128],
                                xt_tiles[kt_][:, n * 512:(n + 1) * 512],
                                start=(kt_ == 0), stop=(kt_ == 7),
                            )
                        nc.vector.tensor_scalar(
                            dst[m][:, n * 512:(n + 1) * 512], p[:],
                            b_sb[:, m:m + 1], None, op0=mybir.AluOpType.add,
                        )

        # ---- phase C: attention ------------------------------------------
        with tc.tile_pool(name="psS", bufs=4, space="PSUM") as psS, \
             tc.tile_pool(name="psC", bufs=2, space="PSUM") as psC, \
             tc.tile_pool(name="exp", bufs=2 * ST + 4) as exp_pool, \
             tc.tile_pool(name="rec", bufs=4) as rec_pool:

            for p_ in range(NPAIRS):
                for q in range(QT_):
                    qs = slice(q * 512, (q + 1) * 512)
                    e0s, e1s = [], []
                    for kt_ in range(ST):
                        ks = slice(kt_ * 128, (kt_ + 1) * 128)
                        ps0 = psS.tile([128, 512], F32, tag="psS")
                        ps1 = psS.tile([128, 512], F32, tag="psS")
                        nc.tensor.matmul(
                            ps0[:], kt_tiles[p_][0:64, ks],
                            qt_tiles[p_][0:64, qs],
                            start=True, stop=True, tile_position=(0, 0),
                        )
                        nc.tensor.matmul(
                            ps1[:], kt_tiles[p_][64:128, ks],
                            qt_tiles[p_][64:128, qs],
                            start=True, stop=True, tile_position=(64, 0),
                        )
                        e0 = exp_pool.tile([128, 512], BF16, tag="exp")
                        e1 = exp_pool.tile([128, 512], BF16, tag="exp")
                        nc.scalar.activation(e0[:], ps0[:], AF.Exp, scale=0.125)
                        nc.scalar.activation(e1[:], ps1[:], AF.Exp, scale=0.125)
                        e0s.append(e0)
                        e1s.append(e1)

                    for (h, es) in ((0, e0s), (1, e1s)):
                        blk = 2 * p_ + h
                        pc = psC.tile([128, 512], F32, tag="psC")
                        for kt_ in range(ST):
                            nc.tensor.matmul(
                                pc[:],
                                vaug_tiles[kt_][:, blk * 128:(blk + 1) * 128],
                                es[kt_][:],
                                start=(kt_ == 0), stop=(kt_ == ST - 1),
                            )
                        rec = rec_pool.tile([128, 512], F32, tag="rec")
                        nc.vector.reciprocal(rec[64:128, :], pc[64:128, :])
                        nc.vector.tensor_mul(
                            ctx_tiles[p_][h * 64:(h + 1) * 64, qs],
                            pc[0:64, :], rec[64:128, :],
                        )

        # ---- phase D: output projection ----------------------------------
        with tc.tile_pool(name="psO", bufs=4, space="PSUM") as psO, \
             tc.tile_pool(name="osb", bufs=4) as osb_pool:
            for st in range(ST):
                ss = slice(st * 128, (st + 1) * 128)
                o_sb = osb_pool.tile([128, D], F32, tag="osb")
                for nh in range(2):
                    po = psO.tile([128, 512], F32, tag="psO")
                    for dt_ in range(4):
                        nc.tensor.matmul(
                            po[:],
                            ctx_tiles[dt_][:, ss],
                            wo_tiles[dt_][:, nh * 512:(nh + 1) * 512],
                            start=(dt_ == 0), stop=(dt_ == 3),
                        )
                    nc.vector.tensor_copy(o_sb[:, nh * 512:(nh + 1) * 512],
                                          po[:])
                nc.sync.dma_start(out[ss, :], o_sb[:])

    nc.finalize()
    return nc


_CACHED = {}


def _get_graph(S):
    if S not in _CACHED:
        _CACHED[S] = build_graph(S)
    return _CACHED[S]


def make_in_maps(x, Wq, bq, Wk, bk, Wv, bv, Wo, bo):
    bf = ml_dtypes.bfloat16
    in_maps = []
    for c in range(8):
        b, hg = c // 2, c % 2
        sl = slice(512 * hg, 512 * (hg + 1))
        in_maps.append({
            "xT": np.ascontiguousarray(x[b].T).astype(bf),
            "wq": np.ascontiguousarray(Wq[:, sl]).astype(bf),
            "wk": np.ascontiguousarray(Wk[:, sl]).astype(bf),
            "wv": np.ascontiguousarray(Wv[:, sl]).astype(bf),
            "wo": np.ascontiguousarray(Wo[sl, :]).astype(bf),
            "bq": np.ascontiguousarray(bq[sl]).reshape(512, 1).astype(np.float32),
            "bk": np.ascontiguousarray(bk[sl]).reshape(512, 1).astype(np.float32),
            "bvr": np.ascontiguousarray(bv[sl]).reshape(1, 512).astype(bf),
            "ones": np.ones((1, 128), dtype=bf),
        })
    return in_maps


def kernel(x, Wq, bq, Wk, bk, Wv, bv, Wo, bo, _trace=False, _tmpdir=None):
    x = np.asarray(x, dtype=np.float32)
    S = x.shape[1]
    nc = _get_graph(S)
    in_maps = make_in_maps(x, np.asarray(Wq), np.asarray(bq), np.asarray(Wk),
                           np.asarray(bk), np.asarray(Wv), np.asarray(bv),
                           np.asarray(Wo), np.asarray(bo))
    res = run_bass_kernel_spmd(
        nc, in_maps, core_ids=list(range(8)), trace=_trace, tmpdir=_tmpdir,
    )
    bo32 = np.asarray(bo, dtype=np.float32)
    outs = [np.asarray(r["out"], dtype=np.float32) for r in res.results]
    full = np.stack([outs[2 * b] + outs[2 * b + 1] + bo32 for b in range(4)])
    kernel.last_results = res
    return full


# revision 10
# speedup vs baseline: 1.2456x; 1.2456x over previous
"""Distributed multi-head attention kernel for one TRN2 chip (8 NeuronCores).

Problem: B=4, S=2048, D=1024, H=16, Dh=64 fp32 attention
    q,k,v = x@W* + b*  (per head)  ->  softmax(q k^T / sqrt(Dh)) v  -> @Wo + bo

Sharding (per the hint): data-parallel over B (4) x tensor-parallel over
head-halves (2) = 8 cores.  Core c = 2*b + hg handles batch b and heads
[8*hg, 8*hg+8) i.e. d_model slice [512*hg, 512*hg+512).  Each core produces
a partial output [2048, 1024] (its 8 heads' contribution through Wo); the
host sums the two partials per batch and adds bo (the unshard step).

Per-core layout trick: everything runs in "features-on-partitions" space.
The host supplies x[b] pre-transposed (xT [1024, 2048]) so Q^T and K^T come
out of the QKV matmuls directly as [d, s], which makes scores^T = K^T.T @ ...
land as [k_seq, q_seq] tiles with k on partitions.  Softmax over k (the
partition axis) is never done directly: we exp() unnormalized, and compute
both ctx^T and the row-sums r in ONE matmul by augmenting V with a block of
ones columns (out rows 0-63 = ctx^T, rows 64-127 = r replicated).  The
normalization (divide by r) happens on ctx^T (64 x 512 tiles), cheap.
Per-head normalization must precede the Wo projection (heads mix there).

Compute dtype: bf16 operands, fp32 PSUM accumulate (rel-err ~1e-3, well
under the 2e-2 gate).  All matmuls run at 1 cycle/row on the PE.
"""

import sys

sys.path.insert(0, "/opt/trn_rl_repo")

import numpy as np
import ml_dtypes

from contextlib import ExitStack

import concourse.bass as bass
import concourse.tile as tile
from concourse import bacc, mybir
from concourse.bass_utils import run_bass_kernel_spmd

BF16 = mybir.dt.bfloat16
F32 = mybir.dt.float32
AF = mybir.ActivationFunctionType


def _install_ntff_hook():
    """Provide antenv.axon_hooks (missing in this image) so that
    run_bass_kernel_spmd(trace=True) can capture NTFF profiles via the
    axon PJRT .so's C ABI."""
    import types, ctypes, contextlib

    if "antenv.axon_hooks" in sys.modules:
        return
    so_path = "/opt/axon/libaxon_pjrt.so"
    mod = types.ModuleType("antenv.axon_hooks")
    _state = {"hook": None}

    def set_axon_ntff_profile_hook(h):
        _state["hook"] = h

    def get_axon_ntff_profile_hook():
        return _state["hook"]

    mod.set_axon_ntff_profile_hook = set_axon_ntff_profile_hook
    mod.get_axon_ntff_profile_hook = get_axon_ntff_profile_hook
    sys.modules["antenv.axon_hooks"] = mod
    import antenv

    antenv.axon_hooks = mod

    try:
        lib = ctypes.CDLL(so_path)
    except OSError:
        return
    if not hasattr(lib, "axon_start_nrt_profile"):
        return
    lib.axon_start_nrt_profile.argtypes = [
        ctypes.POINTER(ctypes.c_int64),
        ctypes.c_size_t,
    ]
    lib.axon_start_nrt_profile.restype = ctypes.c_int64
    lib.axon_stop_nrt_profile.argtypes = [ctypes.c_char_p]
    lib.axon_stop_nrt_profile.restype = ctypes.c_int64

    @contextlib.contextmanager
    def _hook(output_dir, device_ids):
        import jax

        jax.devices()
        if device_ids:
            ids = (ctypes.c_int64 * len(device_ids))(*device_ids)
            rc = lib.axon_start_nrt_profile(ids, len(device_ids))
        else:
            rc = lib.axon_start_nrt_profile(None, 0)
        if rc != 0:
            raise RuntimeError(f"axon_start_nrt_profile rc={rc}")
        try:
            yield
        finally:
            n = lib.axon_stop_nrt_profile(str(output_dir).encode())
            print(f"profile: {n} file(s) written to {output_dir}",
                  file=sys.stderr)

    set_axon_ntff_profile_hook(_hook)


_install_ntff_hook()

D = 1024          # d_model
DC = 512          # per-core d slice (8 heads)
H_CORE = 8        # heads per core
DH = 64           # head dim
NPAIRS = 4        # head pairs per core


def build_graph(S=2048, trace_scopes=False):
    """Build the per-core Bass graph (same graph on all 8 cores)."""
    nc = bacc.Bacc(
        "TRN2",
        target_bir_lowering=False,
        debug=False,
        enable_asserts=False,
        num_devices=8,
    )

    ST = S // 128       # number of 128-seq tiles (16)
    QT_ = S // 512      # number of 512-seq tiles (4)

    xT = nc.dram_tensor("xT", [D, S], BF16, kind="ExternalInput").ap()
    wq = nc.dram_tensor("wq", [D, DC], BF16, kind="ExternalInput").ap()
    wk = nc.dram_tensor("wk", [D, DC], BF16, kind="ExternalInput").ap()
    wv = nc.dram_tensor("wv", [D, DC], BF16, kind="ExternalInput").ap()
    wo = nc.dram_tensor("wo", [DC, D], BF16, kind="ExternalInput").ap()
    bq = nc.dram_tensor("bq", [DC, 1], F32, kind="ExternalInput").ap()
    bk = nc.dram_tensor("bk", [DC, 1], F32, kind="ExternalInput").ap()
    bvr = nc.dram_tensor("bvr", [1, DC], BF16, kind="ExternalInput").ap()
    ones = nc.dram_tensor("ones", [1, 128], BF16, kind="ExternalInput").ap()
    out = nc.dram_tensor("out", [S, D], F32, kind="ExternalOutput").ap()

    with tile.TileContext(nc) as tc, ExitStack() as ctx:
        # ---- persistent pools --------------------------------------------
        qt_pool = ctx.enter_context(tc.tile_pool(name="qt", bufs=4))
        kt_pool = ctx.enter_context(tc.tile_pool(name="kt", bufs=4))
        vaug_pool = ctx.enter_context(tc.tile_pool(name="vaug", bufs=ST))
        ctx_pool = ctx.enter_context(tc.tile_pool(name="ctxT", bufs=4))
        const_pool = ctx.enter_context(tc.tile_pool(name="consts", bufs=1))
        wo_pool = ctx.enter_context(tc.tile_pool(name="wo", bufs=4))

        ones_sb = const_pool.tile([1, 128], BF16, tag="ones")
        nc.sync.dma_start(ones_sb[:], ones[:])
        bvr_sb = const_pool.tile([1, DC], BF16, tag="bvr")
        nc.sync.dma_start(bvr_sb[:], bvr[:])
        bq_sb = const_pool.tile([128, DC // 128], F32, tag="bq")
        nc.sync.dma_start(bq_sb[:], bq.rearrange("(m p) o -> p (m o)", p=128))
        bk_sb = const_pool.tile([128, DC // 128], F32, tag="bk")
        nc.sync.dma_start(bk_sb[:], bk.rearrange("(m p) o -> p (m o)", p=128))

        wo_tiles = []
        for dt_ in range(4):
            t = wo_pool.tile([128, D], BF16, tag="wo")
            nc.sync.dma_start(t[:], wo[dt_ * 128:(dt_ + 1) * 128, :])
            wo_tiles.append(t)

        qt_tiles = [qt_pool.tile([128, S], BF16, tag="qt", name=f"qt{i}") for i in range(4)]
        kt_tiles = [kt_pool.tile([128, S], BF16, tag="kt", name=f"ktt{i}") for i in range(4)]
        vaug_tiles = [vaug_pool.tile([128, H_CORE * 128], BF16, tag="vaug", name=f"vaug{i}")
                      for i in range(ST)]
        ctx_tiles = [ctx_pool.tile([128, S], BF16, tag="ctxT", name=f"ctxT{i}") for i in range(4)]

        # ---- phase B+C: projections interleaved with attention -----------
        # V and QT/KT[m=0] go first; the remaining QT/KT projection groups
        # are sprinkled into the attention loop as PE "filler" so the
        # TensorE never idles long enough for the HAM clock-gate to
        # re-throttle while the ScalarE (exp) is the per-iteration
        # bottleneck.
        xt_cm = tc.tile_pool(name="xt", bufs=8)
        xt_pool = xt_cm.__enter__()
        w_cm = tc.tile_pool(name="wqkv", bufs=24)
        w_pool = w_cm.__enter__()
        psB_cm = tc.tile_pool(name="psB", bufs=2, space="PSUM")
        psB = psB_cm.__enter__()

        xt_tiles = []
        for kt_ in range(8):
            t = xt_pool.tile([128, S], BF16, tag="xt")
            nc.sync.dma_start(t[:], xT[kt_ * 128:(kt_ + 1) * 128, :])
            xt_tiles.append(t)

        wq_tiles, wk_tiles, wv_tiles = [], [], []
        for w_ap, lst in ((wq, wq_tiles), (wk, wk_tiles), (wv, wv_tiles)):
            for kt_ in range(8):
                t = w_pool.tile([128, DC], BF16, tag="wqkv")
                nc.sync.dma_start(t[:], w_ap[kt_ * 128:(kt_ + 1) * 128, :])
                lst.append(t)

        # V in natural [s, d] layout, into Vaug (64 V cols + 64 ones cols
        # per head block); bias via a K=1 ones-row matmul.
        for st in range(ST):
            pv = psB.tile([128, DC], F32, tag="psB")
            for kt_ in range(8):
                nc.tensor.matmul(
                    pv[:],
                    xt_tiles[kt_][:, st * 128:(st + 1) * 128],
                    wv_tiles[kt_][:],
                    start=(kt_ == 0), stop=False,
                )
            nc.tensor.matmul(pv[:], ones_sb[:], bvr_sb[:],
                             start=False, stop=True)
            vt = vaug_tiles[st]
            nc.vector.memset(vt[:], 1.0)
            nc.vector.tensor_copy(
                vt[:].rearrange("p (h w) -> p h w", h=H_CORE)[:, :, 0:64],
                pv[:].rearrange("p (h w) -> p h w", h=H_CORE),
            )

        def proj_group(w_tiles, dst, b_sb, m, n):
            p = psB.tile([128, 512], F32, tag="psB", name=f"psb{m}_{n}")
            for kt_ in range(8):
                nc.tensor.matmul(
                    p[:],
                    w_tiles[kt_][:, m * 128:(m + 1) * 128],
                    xt_tiles[kt_][:, n * 512:(n + 1) * 512],
                    start=(kt_ == 0), stop=(kt_ == 7),
                )
            nc.vector.tensor_scalar(
                dst[m][:, n * 512:(n + 1) * 512], p[:],
                b_sb[:, m:m + 1], None, op0=mybir.AluOpType.add,
            )

        # m=0 projections up front (pair 0 needs them); rest are fillers.
        for n in range(QT_):
            proj_group(wq_tiles, qt_tiles, bq_sb, 0, n)
            proj_group(wk_tiles, kt_tiles, bk_sb, 0, n)
        fillers = []
        for m in range(1, 4):
            for n in range(QT_):
                fillers.append((wq_tiles, qt_tiles, bq_sb, m, n))
                fillers.append((wk_tiles, kt_tiles, bk_sb, m, n))
        filler_i = 0

        # ---- attention ---------------------------------------------------
        psS_cm = tc.tile_pool(name="psS", bufs=2, space="PSUM")
        psS = psS_cm.__enter__()
        psC_cm = tc.tile_pool(name="psC", bufs=2, space="PSUM")
        psC = psC_cm.__enter__()
        exp_cm = tc.tile_pool(name="exp", bufs=4)
        exp_pool = exp_cm.__enter__()
        rec_cm = tc.tile_pool(name="rec", bufs=2)
        rec_pool = rec_cm.__enter__()

        it = 0
        for p_ in range(NPAIRS):
            for q in range(QT_):
                qs = slice(q * 512, (q + 1) * 512)
                pc0 = psC.tile([128, 512], F32, tag="psC", name=f"pc0_{p_}_{q}")
                pc1 = psC.tile([128, 512], F32, tag="psC", name=f"pc1_{p_}_{q}")
                for kt_ in range(ST):
                    ks = slice(kt_ * 128, (kt_ + 1) * 128)
                    ps = psS.tile([128, 1024], F32, tag="psS",
                                  name=f"ps{p_}_{q}_{kt_}")
                    nc.tensor.matmul(
                        ps[:, 0:512], kt_tiles[p_][0:64, ks],
                        qt_tiles[p_][0:64, qs],
                        start=True, stop=True, tile_position=(0, 0),
                    )
                    nc.tensor.matmul(
                        ps[:, 512:1024], kt_tiles[p_][64:128, ks],
                        qt_tiles[p_][64:128, qs],
                        start=True, stop=True, tile_position=(64, 0),
                    )
                    e = exp_pool.tile([128, 1024], BF16, tag="exp",
                                      name=f"e{p_}_{q}_{kt_}")
                    nc.scalar.activation(e[:], ps[:], AF.Exp, scale=0.125)
                    nc.tensor.matmul(
                        pc0[:],
                        vaug_tiles[kt_][:, (2 * p_) * 128:(2 * p_ + 1) * 128],
                        e[:, 0:512],
                        start=(kt_ == 0), stop=(kt_ == ST - 1),
                    )
                    nc.tensor.matmul(
                        pc1[:],
                        vaug_tiles[kt_][:, (2 * p_ + 1) * 128:(2 * p_ + 2) * 128],
                        e[:, 512:1024],
                        start=(kt_ == 0), stop=(kt_ == ST - 1),
                    )
                    it += 1
                    if it % 8 == 0 and filler_i < len(fillers):
                        proj_group(*fillers[filler_i])
                        filler_i += 1

                for h, pc in ((0, pc0), (1, pc1)):
                    rec = rec_pool.tile([128, 512], F32, tag="rec",
                                        name=f"rec{p_}_{q}_{h}")
                    nc.vector.reciprocal(rec[64:128, :], pc[64:128, :])
                    nc.vector.tensor_mul(
                        ctx_tiles[p_][h * 64:(h + 1) * 64, qs],
                        pc[0:64, :], rec[64:128, :],
                    )

        rec_cm.__exit__(None, None, None)
        exp_cm.__exit__(None, None, None)
        psC_cm.__exit__(None, None, None)
        psS_cm.__exit__(None, None, None)
        psB_cm.__exit__(None, None, None)
        w_cm.__exit__(None, None, None)
        xt_cm.__exit__(None, None, None)

        # ---- phase D: output projection ----------------------------------
        with tc.tile_pool(name="psO", bufs=4, space="PSUM") as psO, \
             tc.tile_pool(name="osb", bufs=4) as osb_pool:
            for st in range(ST):
                ss = slice(st * 128, (st + 1) * 128)
                o_sb = osb_pool.tile([128, D], F32, tag="osb")
                for nh in range(2):
                    po = psO.tile([128, 512], F32, tag="psO")
                    for dt_ in range(4):
                        nc.tensor.matmul(
                            po[:],
                            ctx_tiles[dt_][:, ss],
                            wo_tiles[dt_][:, nh * 512:(nh + 1) * 512],
                            start=(dt_ == 0), stop=(dt_ == 3),
                        )
                    nc.vector.tensor_copy(o_sb[:, nh * 512:(nh + 1) * 512],
                                          po[:])
                nc.sync.dma_start(out[ss, :], o_sb[:])

    nc.finalize()
    return nc


_CACHED = {}


def _get_graph(S):
    if S not in _CACHED:
        _CACHED[S] = build_graph(S)
    return _CACHED[S]


def make_in_maps(x, Wq, bq, Wk, bk, Wv, bv, Wo, bo):
    bf = ml_dtypes.bfloat16
    in_maps = []
    for c in range(8):
        b, hg = c // 2, c % 2
        sl = slice(512 * hg, 512 * (hg + 1))
        in_maps.append({
            "xT": np.ascontiguousarray(x[b].T).astype(bf),
            "wq": np.ascontiguousarray(Wq[:, sl]).astype(bf),
            "wk": np.ascontiguousarray(Wk[:, sl]).astype(bf),
            "wv": np.ascontiguousarray(Wv[:, sl]).astype(bf),
            "wo": np.ascontiguousarray(Wo[sl, :]).astype(bf),
            "bq": np.ascontiguousarray(bq[sl]).reshape(512, 1).astype(np.float32),
            "bk": np.ascontiguousarray(bk[sl]).reshape(512, 1).astype(np.float32),
            "bvr": np.ascontiguousarray(bv[sl]).reshape(1, 512).astype(bf),
            "ones": np.ones((1, 128), dtype=bf),
        })
    return in_maps


def kernel(x, Wq, bq, Wk, bk, Wv, bv, Wo, bo, _trace=False, _tmpdir=None):
    x = np.asarray(x, dtype=np.float32)
    S = x.shape[1]
    nc = _get_graph(S)
    in_maps = make_in_maps(x, np.asarray(Wq), np.asarray(bq), np.asarray(Wk),
                           np.asarray(bk), np.asarray(Wv), np.asarray(bv),
                           np.asarray(Wo), np.asarray(bo))
    res = run_bass_kernel_spmd(
        nc, in_maps, core_ids=list(range(8)), trace=_trace, tmpdir=_tmpdir,
    )
    bo32 = np.asarray(bo, dtype=np.float32)
    outs = [np.asarray(r["out"], dtype=np.float32) for r in res.results]
    full = np.stack([outs[2 * b] + outs[2 * b + 1] + bo32 for b in range(4)])
    kernel.last_results = res
    return full


# revision 11
# speedup vs baseline: 1.7507x; 1.4055x over previous
"""Distributed multi-head attention kernel for one TRN2 chip (8 NeuronCores).

Problem: B=4, S=2048, D=1024, H=16, Dh=64 fp32 attention
    q,k,v = x@W* + b*  (per head)  ->  softmax(q k^T / sqrt(Dh)) v  -> @Wo + bo

Sharding (per the hint): data-parallel over B (4) x tensor-parallel over
head-halves (2) = 8 cores.  Core c = 2*b + hg handles batch b and heads
[8*hg, 8*hg+8) i.e. d_model slice [512*hg, 512*hg+512).  Each core produces
a partial output [2048, 1024] (its 8 heads' contribution through Wo); the
host sums the two partials per batch and adds bo (the unshard step).

Per-core layout trick: everything runs in "features-on-partitions" space.
The host supplies x[b] pre-transposed (xT [1024, 2048]) so Q^T and K^T come
out of the QKV matmuls directly as [d, s], which makes scores^T = K^T.T @ ...
land as [k_seq, q_seq] tiles with k on partitions.  Softmax over k (the
partition axis) is never done directly: we exp() unnormalized, and compute
both ctx^T and the row-sums r in ONE matmul by augmenting V with a block of
ones columns (out rows 0-63 = ctx^T, rows 64-127 = r replicated).  The
normalization (divide by r) happens on ctx^T (64 x 512 tiles), cheap.
Per-head normalization must precede the Wo projection (heads mix there).

Compute dtype: bf16 operands, fp32 PSUM accumulate (rel-err ~1e-3, well
under the 2e-2 gate).  All matmuls run at 1 cycle/row on the PE.
"""

import sys

sys.path.insert(0, "/opt/trn_rl_repo")

import numpy as np
import ml_dtypes

from contextlib import ExitStack

import concourse.bass as bass
import concourse.tile as tile
from concourse import bacc, mybir
from concourse.bass_utils import run_bass_kernel_spmd

BF16 = mybir.dt.bfloat16
F32 = mybir.dt.float32
AF = mybir.ActivationFunctionType


def _install_ntff_hook():
    """Provide antenv.axon_hooks (missing in this image) so that
    run_bass_kernel_spmd(trace=True) can capture NTFF profiles via the
    axon PJRT .so's C ABI."""
    import types, ctypes, contextlib

    if "antenv.axon_hooks" in sys.modules:
        return
    so_path = "/opt/axon/libaxon_pjrt.so"
    mod = types.ModuleType("antenv.axon_hooks")
    _state = {"hook": None}

    def set_axon_ntff_profile_hook(h):
        _state["hook"] = h

    def get_axon_ntff_profile_hook():
        return _state["hook"]

    mod.set_axon_ntff_profile_hook = set_axon_ntff_profile_hook
    mod.get_axon_ntff_profile_hook = get_axon_ntff_profile_hook
    sys.modules["antenv.axon_hooks"] = mod
    import antenv

    antenv.axon_hooks = mod

    try:
        lib = ctypes.CDLL(so_path)
    except OSError:
        return
    if not hasattr(lib, "axon_start_nrt_profile"):
        return
    lib.axon_start_nrt_profile.argtypes = [
        ctypes.POINTER(ctypes.c_int64),
        ctypes.c_size_t,
    ]
    lib.axon_start_nrt_profile.restype = ctypes.c_int64
    lib.axon_stop_nrt_profile.argtypes = [ctypes.c_char_p]
    lib.axon_stop_nrt_profile.restype = ctypes.c_int64

    @contextlib.contextmanager
    def _hook(output_dir, device_ids):
        import jax

        jax.devices()
        if device_ids:
            ids = (ctypes.c_int64 * len(device_ids))(*device_ids)
            rc = lib.axon_start_nrt_profile(ids, len(device_ids))
        else:
            rc = lib.axon_start_nrt_profile(None, 0)
        if rc != 0:
            raise RuntimeError(f"axon_start_nrt_profile rc={rc}")
        try:
            yield
        finally:
            n = lib.axon_stop_nrt_profile(str(output_dir).encode())
            print(f"profile: {n} file(s) written to {output_dir}",
                  file=sys.stderr)

    set_axon_ntff_profile_hook(_hook)


_install_ntff_hook()

D = 1024          # d_model
DC = 512          # per-core d slice (8 heads)
H_CORE = 8        # heads per core
DH = 64           # head dim
NPAIRS = 4        # head pairs per core


def build_graph(S=2048, trace_scopes=False):
    """Build the per-core Bass graph (same graph on all 8 cores)."""
    nc = bacc.Bacc(
        "TRN2",
        target_bir_lowering=False,
        debug=False,
        enable_asserts=False,
        num_devices=8,
    )

    ST = S // 128       # number of 128-seq tiles (16)
    QT_ = S // 512      # number of 512-seq tiles (4)

    xT = nc.dram_tensor("xT", [D, S], BF16, kind="ExternalInput").ap()
    wq = nc.dram_tensor("wq", [D, DC], BF16, kind="ExternalInput").ap()
    wk = nc.dram_tensor("wk", [D, DC], BF16, kind="ExternalInput").ap()
    wv = nc.dram_tensor("wv", [D, DC], BF16, kind="ExternalInput").ap()
    wo = nc.dram_tensor("wo", [DC, D], BF16, kind="ExternalInput").ap()
    bq = nc.dram_tensor("bq", [DC, 1], F32, kind="ExternalInput").ap()
    bk = nc.dram_tensor("bk", [DC, 1], F32, kind="ExternalInput").ap()
    bvr = nc.dram_tensor("bvr", [1, DC], BF16, kind="ExternalInput").ap()
    ones = nc.dram_tensor("ones", [1, 128], BF16, kind="ExternalInput").ap()
    out = nc.dram_tensor("out", [S, D], F32, kind="ExternalOutput").ap()

    with tile.TileContext(nc) as tc, ExitStack() as ctx:
        # ---- persistent pools --------------------------------------------
        qt_pool = ctx.enter_context(tc.tile_pool(name="qt", bufs=4))
        kt_pool = ctx.enter_context(tc.tile_pool(name="kt", bufs=4))
        vaug_pool = ctx.enter_context(tc.tile_pool(name="vaug", bufs=ST))
        ctx_pool = ctx.enter_context(tc.tile_pool(name="ctxT", bufs=4))
        const_pool = ctx.enter_context(tc.tile_pool(name="consts", bufs=1))
        wo_pool = ctx.enter_context(tc.tile_pool(name="wo", bufs=4))

        ones_sb = const_pool.tile([1, 128], BF16, tag="ones")
        nc.sync.dma_start(ones_sb[:], ones[:])
        bvr_sb = const_pool.tile([1, DC], BF16, tag="bvr")
        nc.sync.dma_start(bvr_sb[:], bvr[:])
        bq_sb = const_pool.tile([128, DC // 128], F32, tag="bq")
        nc.sync.dma_start(bq_sb[:], bq.rearrange("(m p) o -> p (m o)", p=128))
        bk_sb = const_pool.tile([128, DC // 128], F32, tag="bk")
        nc.sync.dma_start(bk_sb[:], bk.rearrange("(m p) o -> p (m o)", p=128))

        wo_tiles = []
        for dt_ in range(4):
            t = wo_pool.tile([128, D], BF16, tag="wo")
            nc.sync.dma_start(t[:], wo[dt_ * 128:(dt_ + 1) * 128, :])
            wo_tiles.append(t)

        qt_tiles = [qt_pool.tile([128, S], BF16, tag="qt", name=f"qt{i}") for i in range(4)]
        kt_tiles = [kt_pool.tile([128, S], BF16, tag="kt", name=f"ktt{i}") for i in range(4)]
        vaug_tiles = [vaug_pool.tile([128, H_CORE * 128], BF16, tag="vaug", name=f"vaug{i}")
                      for i in range(ST)]
        ctx_tiles = [ctx_pool.tile([128, S], BF16, tag="ctxT", name=f"ctxT{i}") for i in range(4)]

        # ---- phase B+C: projections interleaved with attention -----------
        # V and QT/KT[m=0] go first; the remaining QT/KT projection groups
        # are sprinkled into the attention loop as PE "filler" so the
        # TensorE never idles long enough for the HAM clock-gate to
        # re-throttle while the ScalarE (exp) is the per-iteration
        # bottleneck.
        xt_cm = tc.tile_pool(name="xt", bufs=8)
        xt_pool = xt_cm.__enter__()
        w_cm = tc.tile_pool(name="wqkv", bufs=24)
        w_pool = w_cm.__enter__()
        psB_cm = tc.tile_pool(name="psB", bufs=2, space="PSUM")
        psB = psB_cm.__enter__()

        xt_tiles = []
        for kt_ in range(8):
            t = xt_pool.tile([128, S], BF16, tag="xt")
            nc.sync.dma_start(t[:], xT[kt_ * 128:(kt_ + 1) * 128, :])
            xt_tiles.append(t)

        wq_tiles, wk_tiles, wv_tiles = [], [], []
        for w_ap, lst in ((wq, wq_tiles), (wk, wk_tiles), (wv, wv_tiles)):
            for kt_ in range(8):
                t = w_pool.tile([128, DC], BF16, tag="wqkv")
                nc.sync.dma_start(t[:], w_ap[kt_ * 128:(kt_ + 1) * 128, :])
                lst.append(t)

        # V in natural [s, d] layout, into Vaug (64 V cols + 64 ones cols
        # per head block); bias via a K=1 ones-row matmul.
        for st in range(ST):
            pv = psB.tile([128, DC], F32, tag="psB")
            for kt_ in range(8):
                nc.tensor.matmul(
                    pv[:],
                    xt_tiles[kt_][:, st * 128:(st + 1) * 128],
                    wv_tiles[kt_][:],
                    start=(kt_ == 0), stop=False,
                )
            nc.tensor.matmul(pv[:], ones_sb[:], bvr_sb[:],
                             start=False, stop=True)
            vt = vaug_tiles[st]
            nc.vector.memset(vt[:], 1.0)
            nc.vector.tensor_copy(
                vt[:].rearrange("p (h w) -> p h w", h=H_CORE)[:, :, 0:64],
                pv[:].rearrange("p (h w) -> p h w", h=H_CORE),
            )

        def proj_group(w_tiles, dst, b_sb, m, n):
            p = psB.tile([128, 512], F32, tag="psB", name=f"psb{m}_{n}")
            for kt_ in range(8):
                nc.tensor.matmul(
                    p[:],
                    w_tiles[kt_][:, m * 128:(m + 1) * 128],
                    xt_tiles[kt_][:, n * 512:(n + 1) * 512],
                    start=(kt_ == 0), stop=(kt_ == 7),
                )
            nc.vector.tensor_scalar(
                dst[m][:, n * 512:(n + 1) * 512], p[:],
                b_sb[:, m:m + 1], None, op0=mybir.AluOpType.add,
            )

        # m=0 projections up front (pair 0 needs them); rest are fillers.
        for n in range(QT_):
            proj_group(wq_tiles, qt_tiles, bq_sb, 0, n)
            proj_group(wk_tiles, kt_tiles, bk_sb, 0, n)
        fillers = []
        for m in range(1, 4):
            for n in range(QT_):
                fillers.append((wq_tiles, qt_tiles, bq_sb, m, n))
                fillers.append((wk_tiles, kt_tiles, bk_sb, m, n))
        filler_i = 0

        # ---- attention ---------------------------------------------------
        psS_cm = tc.tile_pool(name="psS", bufs=2, space="PSUM")
        psS = psS_cm.__enter__()
        psC_cm = tc.tile_pool(name="psC", bufs=2, space="PSUM")
        psC = psC_cm.__enter__()
        exp_cm = tc.tile_pool(name="exp", bufs=6)
        exp_pool = exp_cm.__enter__()
        rec_cm = tc.tile_pool(name="rec", bufs=3)
        rec_pool = rec_cm.__enter__()

        it = 0
        for p_ in range(NPAIRS):
            for q in range(QT_):
                qs = slice(q * 512, (q + 1) * 512)
                pc0 = psC.tile([128, 512], F32, tag="psC", name=f"pc0_{p_}_{q}")
                pc1 = psC.tile([128, 512], F32, tag="psC", name=f"pc1_{p_}_{q}")
                for kt_ in range(ST):
                    ks = slice(kt_ * 128, (kt_ + 1) * 128)
                    ps = psS.tile([128, 1024], F32, tag="psS",
                                  name=f"ps{p_}_{q}_{kt_}")
                    nc.tensor.matmul(
                        ps[:, 0:512], kt_tiles[p_][0:64, ks],
                        qt_tiles[p_][0:64, qs],
                        start=True, stop=True, tile_position=(0, 0),
                    )
                    nc.tensor.matmul(
                        ps[:, 512:1024], kt_tiles[p_][64:128, ks],
                        qt_tiles[p_][64:128, qs],
                        start=True, stop=True, tile_position=(64, 0),
                    )
                    e = exp_pool.tile([128, 1024], BF16, tag="exp",
                                      name=f"e{p_}_{q}_{kt_}")
                    nc.scalar.activation(e[:], ps[:], AF.Exp, scale=0.125)
                    nc.tensor.matmul(
                        pc0[:],
                        vaug_tiles[kt_][:, (2 * p_) * 128:(2 * p_ + 1) * 128],
                        e[:, 0:512],
                        start=(kt_ == 0), stop=(kt_ == ST - 1),
                    )
                    nc.tensor.matmul(
                        pc1[:],
                        vaug_tiles[kt_][:, (2 * p_ + 1) * 128:(2 * p_ + 2) * 128],
                        e[:, 512:1024],
                        start=(kt_ == 0), stop=(kt_ == ST - 1),
                    )
                    it += 1
                    if it % 8 == 0 and filler_i < len(fillers):
                        proj_group(*fillers[filler_i])
                        filler_i += 1

                for h, pc in ((0, pc0), (1, pc1)):
                    # evacuate both halves to SBUF quickly (partition-shifted
                    # copies to base 64) so the PSUM bank frees after ~1us
                    # instead of being held through the 4us reciprocal
                    cp = rec_pool.tile([128, 1024], F32, tag="cp",
                                       name=f"cp{p_}_{q}_{h}")
                    nc.vector.tensor_copy(cp[64:128, 0:512], pc[0:64, :])
                    nc.vector.tensor_copy(cp[64:128, 512:1024], pc[64:128, :])
                    rec = rec_pool.tile([128, 512], F32, tag="rec",
                                        name=f"rec{p_}_{q}_{h}")
                    nc.vector.reciprocal(rec[64:128, :], cp[64:128, 512:1024])
                    nc.vector.tensor_mul(
                        ctx_tiles[p_][h * 64:(h + 1) * 64, qs],
                        cp[64:128, 0:512], rec[64:128, :],
                    )

        rec_cm.__exit__(None, None, None)
        exp_cm.__exit__(None, None, None)
        psC_cm.__exit__(None, None, None)
        psS_cm.__exit__(None, None, None)
        psB_cm.__exit__(None, None, None)
        w_cm.__exit__(None, None, None)
        xt_cm.__exit__(None, None, None)

        # ---- phase D: output projection ----------------------------------
        with tc.tile_pool(name="psO", bufs=4, space="PSUM") as psO, \
             tc.tile_pool(name="osb", bufs=4) as osb_pool:
            for st in range(ST):
                ss = slice(st * 128, (st + 1) * 128)
                o_sb = osb_pool.tile([128, D], F32, tag="osb")
                for nh in range(2):
                    po = psO.tile([128, 512], F32, tag="psO")
                    for dt_ in range(4):
                        nc.tensor.matmul(
                            po[:],
                            ctx_tiles[dt_][:, ss],
                            wo_tiles[dt_][:, nh * 512:(nh + 1) * 512],
                            start=(dt_ == 0), stop=(dt_ == 3),
                        )
                    nc.vector.tensor_copy(o_sb[:, nh * 512:(nh + 1) * 512],
                                          po[:])
                nc.sync.dma_start(out[ss, :], o_sb[:])

    nc.finalize()
    return nc


_CACHED = {}


def _get_graph(S):
    if S not in _CACHED:
        _CACHED[S] = build_graph(S)
    return _CACHED[S]


def make_in_maps(x, Wq, bq, Wk, bk, Wv, bv, Wo, bo):
    bf = ml_dtypes.bfloat16
    in_maps = []
    for c in range(8):
        b, hg = c // 2, c % 2
        sl = slice(512 * hg, 512 * (hg + 1))
        in_maps.append({
            "xT": np.ascontiguousarray(x[b].T).astype(bf),
            "wq": np.ascontiguousarray(Wq[:, sl]).astype(bf),
            "wk": np.ascontiguousarray(Wk[:, sl]).astype(bf),
            "wv": np.ascontiguousarray(Wv[:, sl]).astype(bf),
            "wo": np.ascontiguousarray(Wo[sl, :]).astype(bf),
            "bq": np.ascontiguousarray(bq[sl]).reshape(512, 1).astype(np.float32),
            "bk": np.ascontiguousarray(bk[sl]).reshape(512, 1).astype(np.float32),
            "bvr": np.ascontiguousarray(bv[sl]).reshape(1, 512).astype(bf),
            "ones": np.ones((1, 128), dtype=bf),
        })
    return in_maps


def kernel(x, Wq, bq, Wk, bk, Wv, bv, Wo, bo, _trace=False, _tmpdir=None):
    x = np.asarray(x, dtype=np.float32)
    S = x.shape[1]
    nc = _get_graph(S)
    in_maps = make_in_maps(x, np.asarray(Wq), np.asarray(bq), np.asarray(Wk),
                           np.asarray(bk), np.asarray(Wv), np.asarray(bv),
                           np.asarray(Wo), np.asarray(bo))
    res = run_bass_kernel_spmd(
        nc, in_maps, core_ids=list(range(8)), trace=_trace, tmpdir=_tmpdir,
    )
    bo32 = np.asarray(bo, dtype=np.float32)
    outs = [np.asarray(r["out"], dtype=np.float32) for r in res.results]
    full = np.stack([outs[2 * b] + outs[2 * b + 1] + bo32 for b in range(4)])
    kernel.last_results = res
    return full
